# revision 1
# baseline (speedup 1.0000x reference)
"""MinGRU Trainium2 kernel.

Math (linear-space reformulation of the reference's log-space scan; all
quantities are positive so this is numerically safe):
    k = x @ W_z.T ; u = x @ W_h.T
    c_t = sigmoid(-k_t)            # decay coeff (1 - z_t)
    v_t = sigmoid(k_t) * g(u_t)    # input term, g(u) = relu(u) + sigmoid(min(u,0))
    h_t = c_t * h_{t-1} + v_t      # hardware tensor_tensor_scan (mult, add)
    h_0 = g(h0)

Sharding: 8 cores = 4 batches x 2 halves of d_model output channels.
Each core: x[b].T [1024,4096] fp32, weight slices W.T[:, half] [1024,512],
out h.T [512,4096].  Matmuls run as float32r (FP22, full PE rate), scan on
the Vector engine along the time (free) axis, 128 channels per partition.

Per chunk (TC=512 cols), per e-tile (128 channels) so PSUM banks free
incrementally and the PE never stalls:
  PE:     8+8 accumulating MMs into pk{e}/pu{e}
  ACT:    c = sig(-k), r = relu(-u), s = sig(-r) = sig(min(u,0))
  GpSimd: z = 1 - c
  DVE:    g = max(u,0)+s (stt), v = z*g, tensor_tensor_scan
A dozen dummy matmuls at t=0 keep the PE busy through the HAM activity
window so the real matmuls start at 2.4 GHz instead of 1.2.  Measured on
8 axon trn2 cores: ~146-152us HW exec, scale-relative absmax error 1.2e-4
vs the fp64 oracle (fp22 matmul truncation dominates).
"""

import numpy as np

B, T, D = 4, 4096, 1024
EC = 512            # output channels per core
ET = EC // 128      # 4 e-tiles per core
KT = D // 128       # 8 k-tiles
TC = 512            # time chunk (columns per matmul / PSUM bank)
NCHUNK = T // TC    # 8

_CACHED = {}
LAST_RESULT = None


def _build_nc():
    import concourse.bass as bass
    import concourse.bacc as bacc
    import concourse.mybir as mybir
    import concourse.tile as tile

    f32 = mybir.dt.float32
    f32r = mybir.dt.float32r
    AF = mybir.ActivationFunctionType
    OP = mybir.AluOpType

    nc = bacc.Bacc(None, target_bir_lowering=False)

    xT = nc.dram_tensor("xT", [D, T], f32, kind="ExternalInput")
    wz = nc.dram_tensor("wzT", [D, EC], f32, kind="ExternalInput")
    wh = nc.dram_tensor("whT", [D, EC], f32, kind="ExternalInput")
    h0g = nc.dram_tensor("h0g", [128, ET], f32, kind="ExternalInput")
    hT = nc.dram_tensor("hT", [EC, T], f32, kind="ExternalOutput")

    xT_ap = xT[:].rearrange("(kt p) t -> p kt t", p=128).bitcast(f32r)
    wz_ap = wz[:].rearrange("(kt p) e -> p kt e", p=128).bitcast(f32r)
    wh_ap = wh[:].rearrange("(kt p) e -> p kt e", p=128).bitcast(f32r)
    hT_ap = hT[:].rearrange("(et p) t -> p et t", p=128)

    with tile.TileContext(nc) as tc:
        with (
            tc.tile_pool(name="wpool", bufs=1) as wpool,
            tc.tile_pool(name="xpool", bufs=2) as xpool,
            tc.tile_pool(name="work", bufs=2) as work,
            tc.tile_pool(name="hpool", bufs=2) as hpool,
            tc.tile_pool(name="psum", bufs=1, space=bass.MemorySpace.PSUM) as psum,
        ):
            # kt-granular weight tiles so the first matmuls start after
            # ~256KB of DMA, not after the full 4MB weight load.
            wzk = [wpool.tile([128, EC], f32r, tag=f"wz{kt}", name=f"wzk{kt}")
                   for kt in range(KT)]
            whk = [wpool.tile([128, EC], f32r, tag=f"wh{kt}", name=f"whk{kt}")
                   for kt in range(KT)]
            h0_sb = wpool.tile([128, ET], f32, tag="h0")
            warm = wpool.tile([128, TC], mybir.dt.bfloat16, tag="warm")
            nc.vector.memset(warm[:], 0.0)

            sizes = [TC] * NCHUNK
            offs = [sum(sizes[:i]) for i in range(len(sizes))]

            # chunk 0 of x, kt-granular, interleaved with the W_z loads
            x0k = [xpool.tile([128, sizes[0]], f32r, tag=f"x0_{kt}",
                              name=f"x0k{kt}", bufs=1)
                   for kt in range(KT)]
            for kt in range(KT):
                nc.sync.dma_start(out=wzk[kt][:], in_=wz_ap[:, kt, :])
                nc.scalar.dma_start(out=x0k[kt][:],
                                    in_=xT_ap[:, kt, 0:sizes[0]])
            for kt in range(KT):
                nc.sync.dma_start(out=whk[kt][:], in_=wh_ap[:, kt, :])
            nc.gpsimd.dma_start(out=h0_sb[:], in_=h0g[:])

            h_prev = None
            prev_tc = 0
            for ci, (off, tc) in enumerate(zip(offs, sizes)):
                tsl = slice(off, off + tc)
                if ci == 0:
                    xc = x0k
                else:
                    x_sb = xpool.tile([128, KT, tc], f32r, tag="x",
                                      name=f"x_{ci}")
                    nc.sync.dma_start(out=x_sb[:], in_=xT_ap[:, :, tsl])
                    xc = [x_sb[:, kt, :] for kt in range(KT)]

                pk = [psum.tile([128, tc], f32, tag=f"pk{e}", name=f"pk{e}_{ci}")
                      for e in range(ET)]
                pu = [psum.tile([128, tc], f32, tag=f"pu{e}", name=f"pu{e}_{ci}")
                      for e in range(ET)]

                if ci == 0:
                    # HAM pre-warm: keep PE busy during the input DMAs
                    for _ in range(12):
                        nc.tensor.matmul(pk[0][:], warm[:, 0:128],
                                         warm[:, 0:sizes[0]],
                                         start=True, stop=True)

                h = hpool.tile([128, ET, tc], f32, tag="h", name=f"h_{ci}")
                ufirst = (ci == len(sizes) - 1)
                for e in range(ET):
                    esl = slice(e * 128, (e + 1) * 128)
                    groups = ([(pu, whk), (pk, wzk)] if ufirst
                              else [(pk, wzk), (pu, whk)])
                    for dst, wk in groups:
                        for kt in range(KT):
                            nc.tensor.matmul(dst[e][:], wk[kt][:, esl], xc[kt],
                                             start=(kt == 0), stop=(kt == KT - 1))
                for e in range(ET):
                    c = work.tile([128, tc], f32, tag=f"c{e}", name=f"c{e}_{ci}")
                    r = work.tile([128, tc], f32, tag=f"r{e}", name=f"r{e}_{ci}")
                    s = work.tile([128, tc], f32, tag=f"s{e}", name=f"s{e}_{ci}")
                    g = work.tile([128, tc], f32, tag=f"g{e}", name=f"g{e}_{ci}")
                    z = work.tile([128, tc], f32, tag=f"z{e}", name=f"z{e}_{ci}")
                    v = work.tile([128, tc], f32, tag=f"v{e}", name=f"v{e}_{ci}")

                    # ACT: c = sig(-k); r = relu(-u); s = sig(-r) = sig(min(u,0))
                    nc.scalar.activation(c[:], pk[e][:], AF.Sigmoid, scale=-1.0)
                    nc.scalar.activation(r[:], pu[e][:], AF.Relu, scale=-1.0)
                    nc.scalar.activation(s[:], r[:], AF.Sigmoid, scale=-1.0)
                    # GpSimd: z = 1 - c
                    nc.gpsimd.tensor_scalar(z[:], c[:], -1.0, 1.0,
                                            op0=OP.mult, op1=OP.add)
                    # DVE: g = max(u,0) + s ; v = z*g ; scan
                    nc.vector.scalar_tensor_tensor(g[:], pu[e][:], 0.0, s[:],
                                                   op0=OP.max, op1=OP.add)
                    nc.vector.tensor_mul(v[:], z[:], g[:])
                    init = (h0_sb[:, e:e + 1] if ci == 0
                            else h_prev[:, e, prev_tc - 1:prev_tc])
                    nc.vector.tensor_tensor_scan(h[:, e, :], c[:], v[:], init,
                                                 op0=OP.mult, op1=OP.add)
                h_prev = h
                prev_tc = tc
                if ufirst:
                    for e in range(ET):
                        nc.scalar.dma_start(out=hT_ap[:, e, tsl], in_=h[:, e, :])
                else:
                    nc.scalar.dma_start(out=hT_ap[:, :, tsl], in_=h[:])

    nc.compile()
    return nc


def _get_nc():
    if "nc" not in _CACHED:
        _CACHED["nc"] = _build_nc()
    return _CACHED["nc"]


def kernel(x, h0, W_h, W_z, _trace=False):
    global LAST_RESULT
    from concourse import bass_utils

    x = np.asarray(x, np.float32)
    h0 = np.asarray(h0, np.float32)
    W_h = np.asarray(W_h, np.float32)
    W_z = np.asarray(W_z, np.float32)

    # host-side prep: transposes + initial state g(h0)
    gh0 = np.where(h0 >= 0, h0 + np.float32(0.5),
                   1.0 / (1.0 + np.exp(-h0))).astype(np.float32)  # [B,1,D]
    WzT = np.ascontiguousarray(W_z.T)  # [D, D] (in-dim, out-dim)
    WhT = np.ascontiguousarray(W_h.T)

    in_maps = []
    for b in range(B):
        xTb = np.ascontiguousarray(x[b].T)  # [D, T]
        for eh in range(2):
            esl = slice(eh * EC, (eh + 1) * EC)
            h0c = np.ascontiguousarray(
                gh0[b, 0, esl].reshape(ET, 128).T)  # [128, ET]
            in_maps.append({
                "xT": xTb,
                "wzT": np.ascontiguousarray(WzT[:, esl]),
                "whT": np.ascontiguousarray(WhT[:, esl]),
                "h0g": h0c,
            })

    nc = _get_nc()
    try:
        res = bass_utils.run_bass_kernel_spmd(
            nc, in_maps, core_ids=list(range(8)), trace=_trace,
        )
    except Exception:
        # transient NRT_EXEC_UNIT_UNRECOVERABLE has been observed on a
        # first execution; one retry has always succeeded
        res = bass_utils.run_bass_kernel_spmd(
            nc, in_maps, core_ids=list(range(8)), trace=_trace,
        )
    LAST_RESULT = res

    out = np.empty((B, T, D), np.float32)
    for b in range(B):
        for eh in range(2):
            core = b * 2 + eh
            out[b, :, eh * EC:(eh + 1) * EC] = res.results[core]["hT"].T
    return out

